# revision 1
# baseline (speedup 1.0000x reference)
"""Trainium2 Bass kernel for nn_BasicDeconvolutionBlock (sparse transposed conv + BN + ReLU).

Self-contained: hardcodes problem shapes; shards across 8 NeuronCores by
output-site owner; runs one SPMD Bass/Tile program via run_bass_kernel_spmd.

Pipeline per core (out rows [75000c, 75000(c+1))):
  phase A: pairs sorted by (k, local_row); per 128-pair chunk:
      indirect-gather feats rows -> PE transpose -> matmul with W[k] -> C (fp16, DRAM)
  phase B: per 128-row window: indirect-gather its C rows (sorted, padded to
      uniform chunk count), build one-hot SelT via is_equal vs IOTA, matmul-
      accumulate window rows in PSUM; per-channel sum/sumsq stats accumulated
      in PSUM across all windows.
  BN: AllReduce [2,96] stats across 8 cores, scale/bias, normalize+ReLU pass.
"""
import os
import sys
import numpy as np

sys.path.insert(0, "/opt/trn_rl_repo")

N_IN = 200000
N_OUT = 600000
K = 27
P = 150000
C = 96
BN_EPS = 1e-5
NCORES = 8
R_CORE = N_OUT // NCORES          # 75000
NWIN = (R_CORE + 127) // 128      # 586
R_PAD = NWIN * 128                # 75008

_EXEC_TIME_NS = [None]


def _host_prep(in_idx, out_idx):
    """Build per-core index/rowid arrays. Returns dict of numpy arrays + constants."""
    kk = np.repeat(np.arange(K, dtype=np.int64), P)          # [K*P]
    src = in_idx.reshape(-1).astype(np.int64)                # feats row per pair
    dst = out_idx.reshape(-1).astype(np.int64)
    owner = dst // R_CORE
    lrow = dst - owner * R_CORE

    # global sort by (owner, k, lrow)
    key = (owner * K + kk) * (R_PAD + 1) + lrow
    order = np.argsort(key, kind="stable")
    src_s = src[order]
    lrow_s = lrow[order]
    group = (owner * K + kk)[order]                          # sorted too

    counts = np.bincount(group, minlength=NCORES * K).reshape(NCORES, K)
    n_k_max = counts.max(axis=0)                             # [K]
    pad_k = ((n_k_max + 127) // 128) * 128                   # per-k padded size
    chunks_k = (pad_k // 128).astype(np.int64)
    S_pad = int(pad_k.sum())
    n_chunks = int(chunks_k.sum())
    k_chunk_base = np.concatenate([[0], np.cumsum(chunks_k)])[:-1]

    g_start = np.concatenate([[0], np.cumsum(counts.reshape(-1))])  # per (c,k)

    # per-core slot arrays
    A_idx = np.full((NCORES, n_chunks * 128), N_IN, dtype=np.int32)  # pad -> zero row
    slot_lrow = np.full((NCORES, n_chunks * 128), -1, dtype=np.int32)
    slot_off = np.concatenate([[0], np.cumsum(pad_k)])[:-1]          # slot base per k
    for c in range(NCORES):
        for k in range(K):
            g = c * K + k
            n = counts[c, k]
            a = g_start[g]
            base = int(slot_off[k])
            A_idx[c, base:base + n] = src_s[a:a + n]
            slot_lrow[c, base:base + n] = lrow_s[a:a + n]

    # phase B: per (core, window) the slots sorted by lrow.
    # slots within each k-group are lrow-sorted; concatenating k-runs per window.
    NWC_counts = np.zeros((NCORES, NWIN), dtype=np.int64)
    for c in range(NCORES):
        valid = slot_lrow[c] >= 0
        w = slot_lrow[c][valid] // 128
        NWC_counts[c] = np.bincount(w, minlength=NWIN)
    M_w = int(NWC_counts.max())
    NWC = (M_w + 127) // 128                                  # chunks per window
    S_w = NWC * 128

    B_idx = np.zeros((NCORES, NWIN * S_w), dtype=np.int32)    # C row ids (pad -> 0)
    B_rowid = np.full((NCORES, NWIN * S_w), -1.0, dtype=np.float16)
    for c in range(NCORES):
        valid = np.nonzero(slot_lrow[c] >= 0)[0]
        rows = slot_lrow[c][valid]
        o2 = np.argsort(rows, kind="stable")
        pos = valid[o2].astype(np.int32)                      # C row per sorted slot
        rows = rows[o2]
        w = rows // 128
        rel = (rows - w * 128).astype(np.float16)
        # place into window-padded layout
        wc = np.concatenate([[0], np.cumsum(np.bincount(w, minlength=NWIN))])
        for win in range(NWIN):
            a, b = wc[win], wc[win + 1]
            B_idx[c, win * S_w: win * S_w + (b - a)] = pos[a:b]
            B_rowid[c, win * S_w: win * S_w + (b - a)] = rel[a:b]

    # device layouts: partition-major [128, cols]
    def pmaj(arr, ncols):
        return np.ascontiguousarray(
            arr.reshape(ncols, 128).T)

    prep = {
        "S_pad": S_pad, "n_chunks": n_chunks, "NWC": NWC,
        "chunks_k": chunks_k, "k_chunk_base": k_chunk_base,
        "A_idx": [pmaj(A_idx[c], n_chunks) for c in range(NCORES)],
        "B_idx": [pmaj(B_idx[c], NWIN * NWC) for c in range(NCORES)],
        "B_rowid": [pmaj(B_rowid[c], NWIN * NWC) for c in range(NCORES)],
    }
    return prep


def _build(prep):
    import concourse.bass as bass
    import concourse.bacc as bacc
    import concourse.mybir as mybir
    import concourse.tile as tile

    n_chunks = prep["n_chunks"]
    NWC = prep["NWC"]
    chunks_k = prep["chunks_k"]
    k_chunk_base = prep["k_chunk_base"]
    S_pad = prep["S_pad"]

    f16 = mybir.dt.float16
    f32 = mybir.dt.float32
    i32 = mybir.dt.int32

    nc = bacc.Bacc("TRN2", target_bir_lowering=False, debug=False,
                   num_devices=NCORES)
    feats = nc.dram_tensor("feats", [N_IN + 1, C], f16, kind="ExternalInput")
    wmat = nc.dram_tensor("wmat", [C, K * C], f16, kind="ExternalInput")
    a_idx = nc.dram_tensor("a_idx", [128, n_chunks], i32, kind="ExternalInput")
    b_idx = nc.dram_tensor("b_idx", [128, NWIN * NWC], i32, kind="ExternalInput")
    b_rowid = nc.dram_tensor("b_rowid", [128, NWIN * NWC], f16, kind="ExternalInput")
    consts = nc.dram_tensor("consts", [128, 384], f16, kind="ExternalInput")
    gb = nc.dram_tensor("gb", [1, 2 * C], f32, kind="ExternalInput")
    y = nc.dram_tensor("y", [R_PAD, C], f32, kind="ExternalOutput")

    cdram = nc.dram_tensor("cdram", [S_pad, C], f16)
    outp = nc.dram_tensor("outp", [R_PAD, C], f32)
    cc_in = nc.dram_tensor("cc_in", [1, 2 * C], f32)
    cc_out = nc.dram_tensor("cc_out", [1, 2 * C], f32, addr_space="Shared")

    CB = 4  # C-write batching (chunks per DMA)

    with tile.TileContext(nc) as tc:
        with (
            tc.tile_pool(name="const", bufs=1) as cp,
            tc.tile_pool(name="sb", bufs=12) as sb,
            tc.tile_pool(name="sb2", bufs=12) as sb2,
            tc.tile_pool(name="sbig", bufs=3) as sbig,
            tc.tile_pool(name="ps_t", bufs=2, space="PSUM") as ps_t,
            tc.tile_pool(name="ps_c", bufs=2, space="PSUM") as ps_c,
            tc.tile_pool(name="ps_w", bufs=2, space="PSUM") as ps_w,
            tc.tile_pool(name="ps_s", bufs=1, space="PSUM") as ps_s,
        ):
            # constants
            w_t = cp.tile([C, K * C], f16)
            nc.sync.dma_start(out=w_t[:], in_=wmat[:])
            cst = cp.tile([128, 384], f16)
            nc.sync.dma_start(out=cst[:], in_=consts[:])
            ident = cst[:, 0:128]          # identity 128x128
            iota = cst[:, 128:256]         # IOTA[p, i] = i
            ones_t = cst[:, 256:257]       # ones column [128,1] f16
            stats_ps = ps_s.tile([1, 2 * C], f32, space="PSUM", tag="stats")
            ones_row = cp.tile([1, 128], f32)
            nc.vector.memset(ones_row[:], 1.0)
            a_it = cp.tile([128, n_chunks], i32)
            nc.sync.dma_start(out=a_it[:], in_=a_idx[:])
            b_it = cp.tile([128, NWIN * NWC], i32)
            nc.sync.dma_start(out=b_it[:], in_=b_idx[:])
            b_rt = cp.tile([128, NWIN * NWC], f16)
            nc.sync.dma_start(out=b_rt[:], in_=b_rowid[:])

            # ---------------- phase A ----------------
            cstage = None
            for k in range(K):
                for j in range(int(chunks_k[k])):
                    ch = int(k_chunk_base[k]) + j
                    g = sb.tile([128, C], f16, tag="g")
                    nc.gpsimd.indirect_dma_start(
                        out=g[:], out_offset=None, in_=feats[:],
                        in_offset=bass.IndirectOffsetOnAxis(
                            ap=a_it[:, ch:ch + 1], axis=0),
                    )
                    gt_ps = ps_t.tile([C, 128], f16, space="PSUM", tag="gtp")
                    nc.tensor.transpose(out=gt_ps[:], in_=g[:], identity=ident)
                    gt = sb.tile([C, 128], f16, tag="gt")
                    nc.scalar.copy(out=gt[:], in_=gt_ps[:])
                    c_ps = ps_c.tile([128, C], f32, space="PSUM", tag="cp")
                    nc.tensor.matmul(out=c_ps[:], lhsT=gt[:],
                                     rhs=w_t[:, k * C:(k + 1) * C],
                                     start=True, stop=True)
                    if ch % CB == 0:
                        cstage = sbig.tile([128, CB, C], f16, tag="cst")
                    nc.vector.tensor_copy(out=cstage[:, ch % CB, :], in_=c_ps[:])
                    if ch % CB == CB - 1:
                        c0 = (ch - (CB - 1)) * 128
                        nc.sync.dma_start(
                            out=cdram[c0:c0 + CB * 128, :].rearrange(
                                "(b p) c -> p b c", p=128),
                            in_=cstage[:])
            # (n_chunks is a multiple of CB only if chunks_k sums align; handle tail)
            rem = n_chunks % CB
            if rem:
                c0 = (n_chunks - rem) * 128
                nc.sync.dma_start(
                    out=cdram[c0:c0 + rem * 128, :].rearrange(
                        "(b p) c -> p b c", p=128),
                    in_=cstage[:, :rem, :])

            # ---------------- phase B ----------------
            for w in range(NWIN):
                win_ps = ps_w.tile([128, C], f32, space="PSUM", tag="win")
                for j in range(NWC):
                    col = w * NWC + j
                    cg = sb2.tile([128, C], f16, tag="cg")
                    nc.gpsimd.indirect_dma_start(
                        out=cg[:], out_offset=None, in_=cdram[:],
                        in_offset=bass.IndirectOffsetOnAxis(
                            ap=b_it[:, col:col + 1], axis=0),
                    )
                    selt = sb2.tile([128, 128], f16, tag="selt")
                    nc.vector.tensor_tensor(
                        out=selt[:],
                        in0=b_rt[:, col:col + 1].to_broadcast([128, 128]),
                        in1=iota,
                        op=mybir.AluOpType.is_equal,
                    )
                    nc.tensor.matmul(out=win_ps[:], lhsT=selt[:], rhs=cg[:],
                                     start=(j == 0), stop=(j == NWC - 1))
                win_sb = sb2.tile([128, C], f32, tag="winsb")
                nc.vector.tensor_copy(out=win_sb[:], in_=win_ps[:])
                nc.sync.dma_start(out=outp[w * 128:(w + 1) * 128, :], in_=win_sb[:])
                # stats
                win_h = sb2.tile([128, C], f16, tag="winh")
                nc.scalar.copy(out=win_h[:], in_=win_ps[:])
                sq_h = sb2.tile([128, C], f16, tag="sqh")
                nc.vector.tensor_mul(out=sq_h[:], in0=win_h[:], in1=win_h[:])
                nc.tensor.matmul(out=stats_ps[:, 0:C], lhsT=ones_t, rhs=win_h[:],
                                 start=(w == 0), stop=(w == NWIN - 1),
                                 skip_group_check=True)
                nc.tensor.matmul(out=stats_ps[:, C:2 * C], lhsT=ones_t, rhs=sq_h[:],
                                 start=(w == 0), stop=(w == NWIN - 1),
                                 skip_group_check=True)

            # stats -> allreduce
            st_sb = sb2.tile([1, 2 * C], f32)
            nc.vector.tensor_copy(out=st_sb[:], in_=stats_ps[:, :])
            nc.sync.dma_start(out=cc_in[:], in_=st_sb[:])
            nc.gpsimd.collective_compute(
                "AllReduce", mybir.AluOpType.add,
                replica_groups=[list(range(NCORES))],
                ins=[cc_in[:]], outs=[cc_out[:]],
            )
            st2 = sb2.tile([1, 2 * C], f32)
            nc.sync.dma_start(out=st2[:], in_=cc_out[:])
            gb_t = sb2.tile([1, 2 * C], f32)
            nc.sync.dma_start(out=gb_t[:], in_=gb[:])

            # scale = gamma * rsqrt(var+eps); bias = beta - mean*scale  (on [1, C])
            mean = sb2.tile([1, C], f32)
            nc.scalar.mul(out=mean[:], in_=st2[:, 0:C], mul=1.0 / N_OUT)
            ex2 = sb2.tile([1, C], f32)
            nc.scalar.mul(out=ex2[:], in_=st2[:, C:2 * C], mul=1.0 / N_OUT)
            m2 = sb2.tile([1, C], f32)
            nc.vector.tensor_mul(out=m2[:], in0=mean[:], in1=mean[:])
            var = sb2.tile([1, C], f32)
            nc.vector.tensor_sub(out=var[:], in0=ex2[:], in1=m2[:])
            eps_t = sb2.tile([1, 1], f32)
            nc.vector.memset(eps_t[:], BN_EPS)
            std = sb2.tile([1, C], f32)
            nc.scalar.activation(out=std[:], in_=var[:],
                                 func=mybir.ActivationFunctionType.Sqrt,
                                 bias=eps_t[:])
            rstd = sb2.tile([1, C], f32)
            nc.vector.reciprocal(out=rstd[:], in_=std[:])
            scale = sb2.tile([1, C], f32)
            nc.vector.tensor_mul(out=scale[:], in0=gb_t[:, 0:C], in1=rstd[:])
            nbias = sb2.tile([1, C], f32)
            nc.vector.tensor_mul(out=nbias[:], in0=mean[:], in1=scale[:])
            bias = sb2.tile([1, C], f32)
            nc.vector.tensor_sub(out=bias[:], in0=gb_t[:, C:2 * C], in1=nbias[:])

            # broadcast scale/bias to [128, C] via outer product with ones col
            sc_ps = ps_s.tile([128, 2 * C], f32, space="PSUM", tag="scps")
            nc.tensor.matmul(out=sc_ps[:, 0:C], lhsT=ones_row[:], rhs=scale[:],
                             start=True, stop=True, skip_group_check=True)
            nc.tensor.matmul(out=sc_ps[:, C:2 * C], lhsT=ones_row[:], rhs=bias[:],
                             start=True, stop=True, skip_group_check=True)
            sc_t = cp.tile([128, 2 * C], f32)
            nc.vector.tensor_copy(out=sc_t[:], in_=sc_ps[:])

            # ---------------- phase C: normalize + relu ----------------
            NB = 8
            for s in range(0, NWIN, NB):
                nb = min(NB, NWIN - s)
                o_t = sbig.tile([128, NB, C], f32, tag="ot")
                nc.sync.dma_start(
                    out=o_t[:, :nb, :],
                    in_=outp[s * 128:(s + nb) * 128, :].rearrange(
                        "(b p) c -> p b c", p=128))
                for b in range(nb):
                    nc.vector.tensor_mul(out=o_t[:, b, :], in0=o_t[:, b, :],
                                         in1=sc_t[:, 0:C])
                    nc.vector.tensor_add(out=o_t[:, b, :], in0=o_t[:, b, :],
                                         in1=sc_t[:, C:2 * C])
                y_t = sbig.tile([128, NB, C], f32, tag="yt")
                nc.scalar.activation(out=y_t[:, :nb, :], in_=o_t[:, :nb, :],
                                     func=mybir.ActivationFunctionType.Relu)
                nc.sync.dma_start(
                    out=y[s * 128:(s + nb) * 128, :].rearrange(
                        "(b p) c -> p b c", p=128),
                    in_=y_t[:, :nb, :])
    nc.compile()
    return nc


def kernel(**inputs):
    feats = np.asarray(inputs["feats"], dtype=np.float32)
    in_idx = np.asarray(inputs["in_idx"])
    out_idx = np.asarray(inputs["out_idx"])
    weight = np.asarray(inputs["weight"], dtype=np.float32)
    gamma = np.asarray(inputs["gamma"], dtype=np.float32)
    beta = np.asarray(inputs["beta"], dtype=np.float32)

    from concourse.bass_utils import run_bass_kernel_spmd

    prep = _host_prep(in_idx, out_idx)
    nc = _build(prep)

    feats_dev = np.zeros((N_IN + 1, C), dtype=np.float16)
    feats_dev[:N_IN] = feats.astype(np.float16)
    wdev = np.ascontiguousarray(
        weight.astype(np.float16).transpose(1, 0, 2).reshape(C, K * C))
    consts = np.zeros((128, 384), dtype=np.float16)
    consts[:, 0:128] = np.eye(128, dtype=np.float16)
    consts[:, 128:256] = np.arange(128, dtype=np.float16)[None, :]
    consts[:, 256] = 1.0
    gb = np.concatenate([gamma, beta]).astype(np.float32)[None, :]

    in_maps = []
    for c in range(NCORES):
        in_maps.append({
            "feats": feats_dev, "wmat": wdev, "consts": consts, "gb": gb,
            "a_idx": prep["A_idx"][c], "b_idx": prep["B_idx"][c],
            "b_rowid": prep["B_rowid"][c],
        })

    trace = bool(os.environ.get("BASS_KERNEL_TRACE"))
    if trace:
        try:
            _install_trace_shim()
        except Exception as e:
            print(f"trace shim unavailable ({e}); running untraced", file=sys.stderr)
            trace = False
    res = run_bass_kernel_spmd(nc, in_maps, core_ids=list(range(NCORES)),
                               trace=trace)
    if trace:
        _EXEC_TIME_NS[0] = res.exec_time_ns
    y = np.concatenate([res.results[c]["y"][:R_CORE] for c in range(NCORES)],
                       axis=0)
    return y.astype(np.float32)


def _install_trace_shim():
    """Register the NTFF profile hook (missing antenv.axon_hooks on this image)
    and neuter the S3 artifact upload so trace=True works under axon."""
    import types
    if "antenv.axon_hooks" not in sys.modules:
        mod = types.ModuleType("antenv.axon_hooks")
        mod._hook = None
        mod.set_axon_ntff_profile_hook = lambda h: setattr(mod, "_hook", h)
        mod.get_axon_ntff_profile_hook = lambda: mod._hook
        sys.modules["antenv.axon_hooks"] = mod
        sys.path.insert(0, "/root/.axon_site/trn_agent_boot")
        from trn_boot import _ntff_profile_via_ctypes
        mod._hook = _ntff_profile_via_ctypes("/opt/axon/libaxon_pjrt.so")
    import concourse.bass_utils as bu
    bu.upload_artifacts = lambda tmpdir: f"file://{tmpdir}"



# revision 9
# speedup vs baseline: 1.5788x; 1.5788x over previous
"""Trainium2 Bass kernel for nn_BasicDeconvolutionBlock (sparse transposed conv + BN + ReLU).

Self-contained: hardcodes problem shapes; shards across 8 NeuronCores by
output-site owner; runs one SPMD Bass/Tile program via run_bass_kernel_spmd.

Layout (per core): a slot grid [128 lanes x n_chunks chunk-cols] split into
27 k-regions (chunk-col j uses weight W[chunk_k[j]]).  A "stripe" = SS
consecutive chunk-cols in one lane = SS pairs of one (window, k) run, written
by phase A at consecutive addresses of the partition-major cdram
[128, n_chunks*C].  Phase B gathers each output window's <=128 stripes with a
single 128-descriptor indirect DMA (1.7KB per descriptor instead of 192B),
then scatter-adds them into PSUM via one-hot matmuls (SS per window).

Phase A: per chunk-col: 128-descriptor indirect gather of feats rows ->
  PE transpose -> matmul with W[k] -> staged fp16 -> contiguous cdram write.
BN: per-window sum/sumsq accumulated in PSUM, AllReduce [2,96] over 8 cores.
Phase C: normalize+ReLU from partition-major outp, y written partition-major
  and unpermuted on host.
"""
import os
import sys
import numpy as np

sys.path.insert(0, "/opt/trn_rl_repo")

N_IN = 200000
N_OUT = 600000
K = 27
P = 150000
C = 96
BN_EPS = 1e-5
NCORES = 8
R_CORE = N_OUT // NCORES          # 75000
NWIN = (R_CORE + 127) // 128      # 586
R_PAD = NWIN * 128                # 75008

_EXEC_TIME_NS = [None]


def _host_prep(in_idx, out_idx, ss_init=8):
    """Stripe-grid layout; see module docstring."""
    kk = np.repeat(np.arange(K, dtype=np.int64), in_idx.shape[1])
    src = in_idx.reshape(-1).astype(np.int64)
    dst = out_idx.reshape(-1).astype(np.int64)
    owner = dst // R_CORE
    lrow = dst - owner * R_CORE

    key = (owner * K + kk) * (R_PAD + 1) + lrow
    order = np.argsort(key, kind="stable")
    src_s = src[order]
    lrow_s = lrow[order]
    group = (owner * K + kk)[order]
    g_start = np.concatenate(
        [[0], np.cumsum(np.bincount(group, minlength=NCORES * K))])

    for SS in range(ss_init, 16):
        out = _try_layout(SS, src_s, lrow_s, g_start)
        if out is not None:
            return out
    raise RuntimeError("no SS fits")


def _try_layout(SS, src_s, lrow_s, g_start):
    stripes_ck = np.zeros((NCORES, K), dtype=np.int64)
    win_stripes = np.zeros((NCORES, NWIN), dtype=np.int64)
    runs = {}
    for c in range(NCORES):
        for k in range(K):
            a, b = g_start[c * K + k], g_start[c * K + k + 1]
            w = lrow_s[a:b] // 128
            cnt = np.bincount(w, minlength=NWIN)
            st = (cnt + SS - 1) // SS
            runs[(c, k)] = (a, cnt, st)
            stripes_ck[c, k] = st.sum()
            win_stripes[c] += st
    if win_stripes.max() > 128:
        return None
    stripes_k_pad = ((stripes_ck.max(axis=0) + 127) // 128) * 128
    cg_k = stripes_k_pad // 128
    ck = cg_k * SS
    cb = np.concatenate([[0], np.cumsum(ck)])
    n_chunks = int(cb[-1])
    chunk_k = np.repeat(np.arange(K), ck).astype(np.int64)
    nspc = n_chunks // SS

    A_idx = np.full((NCORES, 128, n_chunks), N_IN, dtype=np.int32)
    b_super = np.zeros((NCORES, 128, NWIN), dtype=np.int32)
    b_rt = np.full((NCORES, 128, NWIN * SS), -1.0, dtype=np.float16)

    for c in range(NCORES):
        wfill = np.zeros(NWIN, dtype=np.int64)
        for k in range(K):
            a, cnt, st = runs[(c, k)]
            woff = np.concatenate([[0], np.cumsum(cnt)])
            soff = np.concatenate([[0], np.cumsum(st)])
            base_col = cb[k]
            for w in np.nonzero(cnt)[0]:
                n = int(cnt[w])
                ns = int(st[w])
                pa = a + woff[w]
                rows = lrow_s[pa:pa + n] - w * 128
                srcs = src_s[pa:pa + n]
                t0 = soff[w]
                ts = np.arange(t0, t0 + ns)
                lanes = ts % 128
                cgs = ts // 128
                j0 = base_col + cgs * SS
                sidx = np.arange(n)
                st_i = sidx // SS
                sub = sidx % SS
                A_idx[c, lanes[st_i], j0[st_i] + sub] = srcs
                wslot = wfill[w] + np.arange(ns)
                b_super[c, wslot, w] = lanes * nspc + (j0 // SS)
                fl = np.zeros((ns, SS), dtype=np.float16) - 1.0
                fl[st_i, sub] = rows.astype(np.float16)
                b_rt[c, wslot[:, None],
                     w * SS + np.arange(SS)[None, :]] = fl
                wfill[w] += ns
    return {
        "SS": SS, "n_chunks": n_chunks, "chunk_k": chunk_k,
        "A_idx": A_idx, "b_super": b_super, "b_rt": b_rt,
    }


def _build(prep):
    import concourse.bass as bass
    import concourse.bacc as bacc
    import concourse.mybir as mybir
    import concourse.tile as tile

    n_chunks = prep["n_chunks"]
    SS = prep["SS"]
    chunk_k = prep["chunk_k"]

    f16 = mybir.dt.float16
    f32 = mybir.dt.float32
    i32 = mybir.dt.int32

    nc = bacc.Bacc("TRN2", target_bir_lowering=False, debug=False,
                   num_devices=NCORES)
    feats = nc.dram_tensor("feats", [N_IN + 1, C], f16, kind="ExternalInput")
    wmat = nc.dram_tensor("wmat", [C, K * C], f16, kind="ExternalInput")
    a_idx = nc.dram_tensor("a_idx", [128, n_chunks], i32, kind="ExternalInput")
    b_sup = nc.dram_tensor("b_sup", [128, NWIN], i32, kind="ExternalInput")
    b_rowid = nc.dram_tensor("b_rowid", [128, NWIN * SS], f16,
                             kind="ExternalInput")
    consts = nc.dram_tensor("consts", [128, 384], f16, kind="ExternalInput")
    gb = nc.dram_tensor("gb", [1, 2 * C], f32, kind="ExternalInput")
    y = nc.dram_tensor("y", [128, NWIN * C], f32, kind="ExternalOutput")

    cdram = nc.dram_tensor("cdram", [128, n_chunks * C], f16)
    outp = nc.dram_tensor("outp", [128, NWIN * C], f32)
    cc_in = nc.dram_tensor("cc_in", [1, 2 * C], f32)
    cc_out = nc.dram_tensor("cc_out", [1, 2 * C], f32, addr_space="Shared")

    cview = cdram.rearrange("p (r c) -> (p r) c", c=SS * C)

    CB = 8   # chunk-cols per cdram write
    WB = 4   # windows per outp write

    with tile.TileContext(nc) as tc:
        with (
            tc.tile_pool(name="const", bufs=1) as cp,
            tc.tile_pool(name="sb", bufs=12) as sb,
            tc.tile_pool(name="sb2", bufs=12) as sb2,
            tc.tile_pool(name="sbig", bufs=3) as sbig,
            tc.tile_pool(name="ps_t", bufs=2, space="PSUM") as ps_t,
            tc.tile_pool(name="ps_c", bufs=2, space="PSUM") as ps_c,
            tc.tile_pool(name="ps_w", bufs=2, space="PSUM") as ps_w,
            tc.tile_pool(name="ps_s", bufs=1, space="PSUM") as ps_s,
        ):
            # constants
            w_t = cp.tile([C, K * C], f16)
            nc.sync.dma_start(out=w_t[:], in_=wmat[:])
            cst = cp.tile([128, 384], f16)
            nc.sync.dma_start(out=cst[:], in_=consts[:])
            ident = cst[:, 0:128]          # identity 128x128
            iota = cst[:, 128:256]         # IOTA[p, i] = i
            ones_t = cst[:, 256:257]       # ones column [128,1] f16
            stats_ps = ps_s.tile([1, 2 * C], f32, space="PSUM", tag="stats")
            ones_row = cp.tile([1, 128], f32)
            nc.vector.memset(ones_row[:], 1.0)
            a_it = cp.tile([128, n_chunks], i32)
            nc.sync.dma_start(out=a_it[:], in_=a_idx[:])
            b_st = cp.tile([128, NWIN], i32)
            nc.sync.dma_start(out=b_st[:], in_=b_sup[:])
            b_rt = cp.tile([128, NWIN * SS], f16)
            nc.sync.dma_start(out=b_rt[:], in_=b_rowid[:])

            # ---------------- phase A ----------------
            cstage = None
            for ch in range(n_chunks):
                k = int(chunk_k[ch])
                g = sb.tile([128, C], f16, tag="g")
                nc.gpsimd.indirect_dma_start(
                    out=g[:], out_offset=None, in_=feats[:],
                    in_offset=bass.IndirectOffsetOnAxis(
                        ap=a_it[:, ch:ch + 1], axis=0),
                )
                gt_ps = ps_t.tile([C, 128], f16, space="PSUM", tag="gtp")
                nc.tensor.transpose(out=gt_ps[:], in_=g[:], identity=ident)
                gt = sb.tile([C, 128], f16, tag="gt")
                nc.scalar.copy(out=gt[:], in_=gt_ps[:])
                c_ps = ps_c.tile([128, C], f32, space="PSUM", tag="cp")
                nc.tensor.matmul(out=c_ps[:], lhsT=gt[:],
                                 rhs=w_t[:, k * C:(k + 1) * C],
                                 start=True, stop=True)
                if ch % CB == 0:
                    cstage = sbig.tile([128, CB, C], f16, tag="cst")
                nc.vector.tensor_copy(out=cstage[:, ch % CB, :], in_=c_ps[:])
                if ch % CB == CB - 1:
                    c0 = ch - (CB - 1)
                    nc.sync.dma_start(
                        out=cdram[:, c0 * C:(c0 + CB) * C],
                        in_=cstage[:].rearrange("p b c -> p (b c)"))
            rem = n_chunks % CB
            if rem:
                c0 = n_chunks - rem
                nc.sync.dma_start(
                    out=cdram[:, c0 * C:(c0 + rem) * C],
                    in_=cstage[:, :rem, :].rearrange("p b c -> p (b c)"))

            # ---------------- phase B ----------------
            wstage = None
            for w in range(NWIN):
                bt = sb2.tile([128, SS * C], f16, tag="bt")
                nc.gpsimd.indirect_dma_start(
                    out=bt[:], out_offset=None, in_=cview[:],
                    in_offset=bass.IndirectOffsetOnAxis(
                        ap=b_st[:, w:w + 1], axis=0),
                )
                win_ps = ps_w.tile([128, C], f32, space="PSUM", tag="win")
                for s in range(SS):
                    col = w * SS + s
                    selt = sb2.tile([128, 128], f16, tag="selt")
                    nc.vector.tensor_tensor(
                        out=selt[:],
                        in0=b_rt[:, col:col + 1].to_broadcast([128, 128]),
                        in1=iota,
                        op=mybir.AluOpType.is_equal,
                    )
                    nc.tensor.matmul(out=win_ps[:], lhsT=selt[:],
                                     rhs=bt[:, s * C:(s + 1) * C],
                                     start=(s == 0), stop=(s == SS - 1))
                if w % WB == 0:
                    wstage = sbig.tile([128, WB, C], f32, tag="wst")
                nc.vector.tensor_copy(out=wstage[:, w % WB, :], in_=win_ps[:])
                if w % WB == WB - 1:
                    w0 = w - (WB - 1)
                    nc.sync.dma_start(
                        out=outp[:, w0 * C:(w0 + WB) * C],
                        in_=wstage[:].rearrange("p b c -> p (b c)"))
                # stats
                win_h = sb2.tile([128, C], f16, tag="winh")
                nc.scalar.copy(out=win_h[:], in_=win_ps[:])
                sq_h = sb2.tile([128, C], f16, tag="sqh")
                nc.vector.tensor_mul(out=sq_h[:], in0=win_h[:], in1=win_h[:])
                nc.tensor.matmul(out=stats_ps[:, 0:C], lhsT=ones_t, rhs=win_h[:],
                                 start=(w == 0), stop=(w == NWIN - 1),
                                 skip_group_check=True)
                nc.tensor.matmul(out=stats_ps[:, C:2 * C], lhsT=ones_t, rhs=sq_h[:],
                                 start=(w == 0), stop=(w == NWIN - 1),
                                 skip_group_check=True)
            remw = NWIN % WB
            if remw:
                w0 = NWIN - remw
                nc.sync.dma_start(
                    out=outp[:, w0 * C:(w0 + remw) * C],
                    in_=wstage[:, :remw, :].rearrange("p b c -> p (b c)"))

            # stats -> allreduce
            st_sb = sb2.tile([1, 2 * C], f32)
            nc.vector.tensor_copy(out=st_sb[:], in_=stats_ps[:, :])
            nc.sync.dma_start(out=cc_in[:], in_=st_sb[:])
            nc.gpsimd.collective_compute(
                "AllReduce", mybir.AluOpType.add,
                replica_groups=[list(range(NCORES))],
                ins=[cc_in[:]], outs=[cc_out[:]],
            )
            st2 = sb2.tile([1, 2 * C], f32)
            nc.sync.dma_start(out=st2[:], in_=cc_out[:])
            gb_t = sb2.tile([1, 2 * C], f32)
            nc.sync.dma_start(out=gb_t[:], in_=gb[:])

            # scale = gamma * rsqrt(var+eps); bias = beta - mean*scale
            mean = sb2.tile([1, C], f32)
            nc.scalar.mul(out=mean[:], in_=st2[:, 0:C], mul=1.0 / N_OUT)
            ex2 = sb2.tile([1, C], f32)
            nc.scalar.mul(out=ex2[:], in_=st2[:, C:2 * C], mul=1.0 / N_OUT)
            m2 = sb2.tile([1, C], f32)
            nc.vector.tensor_mul(out=m2[:], in0=mean[:], in1=mean[:])
            var = sb2.tile([1, C], f32)
            nc.vector.tensor_sub(out=var[:], in0=ex2[:], in1=m2[:])
            eps_t = sb2.tile([1, 1], f32)
            nc.vector.memset(eps_t[:], BN_EPS)
            std = sb2.tile([1, C], f32)
            nc.scalar.activation(out=std[:], in_=var[:],
                                 func=mybir.ActivationFunctionType.Sqrt,
                                 bias=eps_t[:])
            rstd = sb2.tile([1, C], f32)
            nc.vector.reciprocal(out=rstd[:], in_=std[:])
            scale = sb2.tile([1, C], f32)
            nc.vector.tensor_mul(out=scale[:], in0=gb_t[:, 0:C], in1=rstd[:])
            nbias = sb2.tile([1, C], f32)
            nc.vector.tensor_mul(out=nbias[:], in0=mean[:], in1=scale[:])
            bias = sb2.tile([1, C], f32)
            nc.vector.tensor_sub(out=bias[:], in0=gb_t[:, C:2 * C], in1=nbias[:])

            # broadcast scale/bias to [128, C]
            sc_ps = ps_s.tile([128, 2 * C], f32, space="PSUM", tag="scps")
            nc.tensor.matmul(out=sc_ps[:, 0:C], lhsT=ones_row[:], rhs=scale[:],
                             start=True, stop=True, skip_group_check=True)
            nc.tensor.matmul(out=sc_ps[:, C:2 * C], lhsT=ones_row[:], rhs=bias[:],
                             start=True, stop=True, skip_group_check=True)
            sc_t = cp.tile([128, 2 * C], f32)
            nc.vector.tensor_copy(out=sc_t[:], in_=sc_ps[:])

            # ---------------- phase C: normalize + relu ----------------
            NB = 8
            for s0 in range(0, NWIN, NB):
                nb = min(NB, NWIN - s0)
                o_t = sbig.tile([128, NB, C], f32, tag="ot")
                nc.sync.dma_start(
                    out=o_t[:, :nb, :].rearrange("p b c -> p (b c)"),
                    in_=outp[:, s0 * C:(s0 + nb) * C])
                for b in range(nb):
                    nc.vector.tensor_mul(out=o_t[:, b, :], in0=o_t[:, b, :],
                                         in1=sc_t[:, 0:C])
                    nc.vector.tensor_add(out=o_t[:, b, :], in0=o_t[:, b, :],
                                         in1=sc_t[:, C:2 * C])
                y_t = sbig.tile([128, NB, C], f32, tag="yt")
                nc.scalar.activation(out=y_t[:, :nb, :], in_=o_t[:, :nb, :],
                                     func=mybir.ActivationFunctionType.Relu)
                nc.sync.dma_start(
                    out=y[:, s0 * C:(s0 + nb) * C],
                    in_=y_t[:, :nb, :].rearrange("p b c -> p (b c)"))
    nc.compile()
    return nc


def kernel(**inputs):
    feats = np.asarray(inputs["feats"], dtype=np.float32)
    in_idx = np.asarray(inputs["in_idx"])
    out_idx = np.asarray(inputs["out_idx"])
    weight = np.asarray(inputs["weight"], dtype=np.float32)
    gamma = np.asarray(inputs["gamma"], dtype=np.float32)
    beta = np.asarray(inputs["beta"], dtype=np.float32)

    from concourse.bass_utils import run_bass_kernel_spmd

    prep = _host_prep(in_idx, out_idx)
    nc = _build(prep)

    feats_dev = np.zeros((N_IN + 1, C), dtype=np.float16)
    feats_dev[:N_IN] = feats.astype(np.float16)
    wdev = np.ascontiguousarray(
        weight.astype(np.float16).transpose(1, 0, 2).reshape(C, K * C))
    consts = np.zeros((128, 384), dtype=np.float16)
    consts[:, 0:128] = np.eye(128, dtype=np.float16)
    consts[:, 128:256] = np.arange(128, dtype=np.float16)[None, :]
    consts[:, 256] = 1.0
    gb = np.concatenate([gamma, beta]).astype(np.float32)[None, :]

    in_maps = []
    for c in range(NCORES):
        in_maps.append({
            "feats": feats_dev, "wmat": wdev, "consts": consts, "gb": gb,
            "a_idx": np.ascontiguousarray(prep["A_idx"][c]),
            "b_sup": np.ascontiguousarray(prep["b_super"][c]),
            "b_rowid": np.ascontiguousarray(prep["b_rt"][c]),
        })

    trace = bool(os.environ.get("BASS_KERNEL_TRACE"))
    if trace:
        try:
            _install_trace_shim()
        except Exception as e:
            print(f"trace shim unavailable ({e}); running untraced", file=sys.stderr)
            trace = False
    res = run_bass_kernel_spmd(nc, in_maps, core_ids=list(range(NCORES)),
                               trace=trace)
    if trace:
        _EXEC_TIME_NS[0] = res.exec_time_ns
        try:
            _dump_trace_summary(res)
        except Exception as e:
            print(f"trace summary failed: {e}", file=sys.stderr)
    # y is partition-major [128, NWIN*C]: row (w*128+p) = y_pm[p, w, :]
    parts = []
    for c in range(NCORES):
        ypm = res.results[c]["y"].reshape(128, NWIN, C)
        parts.append(ypm.transpose(1, 0, 2).reshape(R_PAD, C)[:R_CORE])
    return np.concatenate(parts, axis=0).astype(np.float32)


def _dump_trace_summary(res, path="/tmp/trace_summary.txt"):
    """Per-engine busy time + top instruction groups from the NTFF insts."""
    from collections import defaultdict
    insts = res.instructions_and_trace[0] if res.instructions_and_trace else []
    if not insts:
        return
    t0 = min(i.timestamp for i in insts)
    t1 = max(i.end_timestamp for i in insts)
    span = max(1, t1 - t0)
    eng_busy = defaultdict(int)
    eng_cnt = defaultdict(int)
    for i in insts:
        e = str(i.engine)
        eng_busy[e] += i.duration
        eng_cnt[e] += 1
    with open(path, "w") as f:
        f.write(f"span: {span} ns  ({t0}..{t1})\n\nengine busy:\n")
        for e in sorted(eng_busy, key=lambda x: -eng_busy[x]):
            f.write(f"  {e:<14} {eng_busy[e]:>12} ns  {100.0*eng_busy[e]/span:6.1f}%"
                    f"  n={eng_cnt[e]}\n")
    print(f"trace summary -> {path}", file=sys.stderr)


def _install_trace_shim():
    """Register the NTFF profile hook (missing antenv.axon_hooks on this image)
    and neuter the S3 artifact upload so trace=True works under axon."""
    import types
    if "antenv.axon_hooks" not in sys.modules:
        mod = types.ModuleType("antenv.axon_hooks")
        mod._hook = None
        mod.set_axon_ntff_profile_hook = lambda h: setattr(mod, "_hook", h)
        mod.get_axon_ntff_profile_hook = lambda: mod._hook
        sys.modules["antenv.axon_hooks"] = mod
        sys.path.insert(0, "/root/.axon_site/trn_agent_boot")
        from trn_boot import _ntff_profile_via_ctypes
        mod._hook = _ntff_profile_via_ctypes("/opt/axon/libaxon_pjrt.so")
    import concourse.bass_utils as bu
    bu.upload_artifacts = lambda tmpdir: f"file://{tmpdir}"
